# revision 1
# baseline (speedup 1.0000x reference)
"""nGPT-style causal attention block on 8 TRN2 NeuronCores.

Sharding: core = (batch b, head-group g); b = core // 4, g = core % 4.
Each core handles 1 batch x 4 heads (e-slice of 512 channels) and produces a
partial output P = (attention_out @ WoutN[:, sl].T).T of shape [DIM, SEQ];
the host sums the 4 head-group partials per batch and transposes.

All FLOPs (weight row/col l2-norms, projections, per-head q/k l2-norm,
qk_scale, causal softmax attention, output projection) run on device.
Host does only slicing / transposition / final partial-sum gather.

Matmuls run as float32r (full-rate fp32 path on the PE, ~1.5e-4 rel err).
Partition-broadcasts are done with SBUF->SBUF DMA; softmax uses no max pass
(scores are bounded by qk_scale * sqrt(dh)); exp runs on 1024-wide chunks.
"""
import numpy as np
from contextlib import ExitStack

import concourse.bacc as bacc
import concourse.tile as tile
from concourse import mybir
from concourse.bass_utils import run_bass_kernel_spmd

DIM = 2048          # model dim (= contraction dim of projections)
SEQ = 2048          # sequence length
B = 2               # batch
HEADS = 16
DH = 128            # head dim
NCORES = 8
HPC = 4             # heads per core
ES = HPC * DH       # 512 channels per core
KT = DIM // 128     # 16 contraction tiles
NCH = SEQ // 512    # 4 chunks of 512
NC2 = SEQ // 1024   # 2 chunks of 1024
ATT_SCALE = float(DH) ** 0.5

f32 = mybir.dt.float32
f32r = mybir.dt.float32r
AF = mybir.ActivationFunctionType
ALU = mybir.AluOpType


def build_program(repeat=1):
    nc = bacc.Bacc("TRN2", target_bir_lowering=False)

    # ---- per-core DRAM I/O ----
    xT_d = nc.dram_tensor("xT", [DIM, SEQ], f32r, kind="ExternalInput")
    wqT_d = nc.dram_tensor("wqT", [HPC, DIM, DH], f32r, kind="ExternalInput")
    wkT_d = nc.dram_tensor("wkT", [HPC, DIM, DH], f32r, kind="ExternalInput")
    wvT_d = nc.dram_tensor("wvT", [DIM, ES], f32r, kind="ExternalInput")
    wqN_d = nc.dram_tensor("wqN", [HPC, 128, DIM], f32, kind="ExternalInput")
    wkN_d = nc.dram_tensor("wkN", [HPC, 128, DIM], f32, kind="ExternalInput")
    wvN_d = nc.dram_tensor("wvN", [HPC, 128, DIM], f32, kind="ExternalInput")
    woT_d = nc.dram_tensor("woT", [ES, DIM], f32r, kind="ExternalInput")
    qs_d = nc.dram_tensor("qs", [128, HPC], f32, kind="ExternalInput")
    tri_d = nc.dram_tensor("tri", [128, 128], f32, kind="ExternalInput")
    onec_d = nc.dram_tensor("onec", [128, 1], f32r, kind="ExternalInput")
    out_d = nc.dram_tensor("out", [DIM, SEQ], f32, kind="ExternalOutput")

    with tile.TileContext(nc) as tc:
      for _rep in range(repeat):
        with ExitStack() as top:
            consts = top.enter_context(tc.tile_pool(name="consts", bufs=1))
            scr = top.enter_context(tc.tile_pool(name="scr", bufs=1, space="DRAM"))
            v_scr = scr.tile([SEQ, ES], f32r)
            q_scr = scr.tile([HPC, DH, SEQ], f32r)
            k_scr = scr.tile([HPC, DH, SEQ], f32r)
            oT_scr = scr.tile([HPC, DH, SEQ], f32r)
            ssr_scr = scr.tile([1, SEQ], f32)
            ar_scr = scr.tile([1, 1024], f32)

            tri_sb = consts.tile([128, 128], f32)
            qs_sb = consts.tile([128, HPC], f32)
            onec_sb = consts.tile([128, 1], f32r)
            se_sb = consts.tile([128, HPC], f32)   # effective qk scale
            wv_rn = consts.tile([128, HPC], f32)
            wq_rn = consts.tile([128, HPC], f32)
            wk_rn = consts.tile([128, HPC], f32)
            ssn = consts.tile([128, HPC], f32)
            nc.sync.dma_start(out=tri_sb, in_=tri_d[:])
            nc.sync.dma_start(out=qs_sb, in_=qs_d[:])
            nc.sync.dma_start(out=onec_sb, in_=onec_d[:])
            nc.vector.tensor_scalar_mul(se_sb, qs_sb, float(DIM))

            # ---- weight row norms (wq, wk, wv) from natural layouts (DVE) ----
            with tc.tile_pool(name="natw", bufs=2) as natw:
                for w_nat_d, rn_tile in ((wvN_d, wv_rn), (wqN_d, wq_rn),
                                         (wkN_d, wk_rn)):
                    for h in range(HPC):
                        nt = natw.tile([128, DIM], f32, tag="nat")
                        nc.sync.dma_start(out=nt, in_=w_nat_d[h])
                        sqn = natw.tile([128, DIM], f32, tag="sqn")
                        nc.vector.tensor_mul(sqn, nt, nt)
                        nc.vector.tensor_reduce(
                            ssn[:, h:h + 1], sqn, axis=mybir.AxisListType.X,
                            op=ALU.add)
                    nc.scalar.activation(rn_tile, ssn, AF.Sqrt)
                    nc.vector.reciprocal(rn_tile, rn_tile)

            # ================= phases with xT resident =================
            with ExitStack() as xctx:
                xpool = xctx.enter_context(tc.tile_pool(name="xpool", bufs=1))
                xt = xpool.tile([128, KT, SEQ], f32r)
                for k in range(KT):
                    nc.sync.dma_start(out=xt[:, k, :],
                                      in_=xT_d[k * 128:(k + 1) * 128, :])

                # ---- phase A: v natural (all heads), spill to DRAM ----
                with tc.tile_pool(name="phA", bufs=1) as phA, \
                     tc.tile_pool(name="phAe", bufs=3) as phAe, \
                     tc.tile_pool(name="phA_ps", bufs=2, space="PSUM") as phA_ps:
                    wvT_sb = phA.tile([128, KT, ES], f32r, tag="wvT")
                    for k in range(KT):
                        nc.sync.dma_start(out=wvT_sb[:, k, :],
                                          in_=wvT_d[k * 128:(k + 1) * 128, :])
                    for t in range(SEQ // 128):
                        pv = phA_ps.tile([128, ES], f32, tag="pv")
                        for k in range(KT):
                            nc.tensor.matmul(
                                pv, xt[:, k, t * 128:(t + 1) * 128],
                                wvT_sb[:, k, :],
                                start=(k == 0), stop=(k == KT - 1))
                        vsb = phAe.tile([128, ES], f32r, tag="vev")
                        nc.vector.tensor_copy(vsb, pv)
                        nc.sync.dma_start(
                            out=v_scr[t * 128:(t + 1) * 128, :], in_=vsb[:])

                # ---- phase B1: q/k projection + norms, spill per head ----
                with tc.tile_pool(name="phB1", bufs=2) as phB1, \
                     tc.tile_pool(name="phB1n", bufs=3) as phB1n, \
                     tc.tile_pool(name="phB1r", bufs=2) as phB1r, \
                     tc.tile_pool(name="pqps", bufs=3, space="PSUM") as pqps, \
                     tc.tile_pool(name="pssps", bufs=4, space="PSUM") as pssps:
                    for h in range(HPC):
                        for dst_scr, wT_dram, rn_w, is_q in (
                                (q_scr, wqT_d, wq_rn, True),
                                (k_scr, wkT_d, wk_rn, False)):
                            wsb = phB1.tile([128, KT, DH], f32r, tag="wT")
                            for k in range(KT):
                                nc.sync.dma_start(
                                    out=wsb[:, k, :],
                                    in_=wT_dram[h, k * 128:(k + 1) * 128, :])
                            qT = phB1.tile([128, SEQ], f32r, tag="qk")
                            ssrow = phB1r.tile([1, SEQ], f32, tag="ssrow")
                            for c in range(NCH):
                                sl = slice(c * 512, (c + 1) * 512)
                                pq = pqps.tile([128, 512], f32, tag="pq")
                                for k in range(KT):
                                    nc.tensor.matmul(
                                        pq, wsb[:, k, :], xt[:, k, sl],
                                        start=(k == 0), stop=(k == KT - 1))
                                # evict with weight-row-norm fold
                                nc.vector.tensor_scalar_mul(
                                    qT[:, sl], pq, rn_w[:, h:h + 1])
                                sq = phB1n.tile([128, 512], f32r, tag="nw")
                                nc.vector.tensor_mul(
                                    sq, qT[:, sl].bitcast(f32),
                                    qT[:, sl].bitcast(f32))
                                pss = pssps.tile([1, 512], f32, tag="pss")
                                nc.tensor.matmul(pss, onec_sb, sq,
                                                 start=True, stop=True)
                                nc.vector.tensor_copy(ssrow[:, sl], pss)
                            # rnorm row: 1/sqrt(ss)
                            nc.scalar.activation(ssrow, ssrow, AF.Sqrt)
                            nc.vector.reciprocal(ssrow, ssrow)
                            nc.sync.dma_start(out=ssr_scr[:], in_=ssrow[:])
                            for c in range(NCH):
                                sl = slice(c * 512, (c + 1) * 512)
                                rbc = phB1n.tile([128, 512], f32, tag="rbc")
                                nc.sync.dma_start(
                                    out=rbc,
                                    in_=ssr_scr[:, sl].to_broadcast([128, 512]))
                                if is_q:
                                    nc.vector.tensor_scalar_mul(
                                        rbc, rbc, se_sb[:, h:h + 1])
                                nc.vector.tensor_mul(
                                    qT[:, sl], qT[:, sl].bitcast(f32), rbc)
                            nc.sync.dma_start(out=dst_scr[h], in_=qT[:])

            # ---- phase B2: attention per head (xT freed) ----
            with tc.tile_pool(name="phB2", bufs=2) as phB2, \
                 tc.tile_pool(name="phB2e", bufs=4) as phB2e, \
                 tc.tile_pool(name="phB2r", bufs=2) as phB2r, \
                 tc.tile_pool(name="pscps", bufs=2, space="PSUM") as pscps, \
                 tc.tile_pool(name="pops", bufs=1, space="PSUM") as pops, \
                 tc.tile_pool(name="psrps", bufs=1, space="PSUM") as psrps:
                for h in range(HPC):
                    qT = phB2.tile([128, SEQ], f32r, tag="qT")
                    nc.sync.dma_start(out=qT, in_=q_scr[h])
                    kT = phB2.tile([128, SEQ], f32r, tag="kT")
                    nc.sync.dma_start(out=kT, in_=k_scr[h])
                    vh = phB2.tile([128, SEQ // 128, DH], f32r, tag="vh")
                    for t in range(SEQ // 128):
                        nc.sync.dma_start(
                            out=vh[:, t, :],
                            in_=v_scr[t * 128:(t + 1) * 128,
                                      h * DH:(h + 1) * DH])
                    for c2 in range(NC2):
                        nj = 8 * c2 + 8
                        po = pops.tile([128, 1024], f32, tag="po")
                        psr = psrps.tile([1, 1024], f32, tag="psr")
                        for J in range(nj):
                            psc = pscps.tile([128, 1024], f32, tag="psc")
                            for half in range(2):
                                isl = slice(c2 * 1024 + half * 512,
                                            c2 * 1024 + (half + 1) * 512)
                                nc.tensor.matmul(
                                    psc[:, half * 512:(half + 1) * 512],
                                    kT[:, J * 128:(J + 1) * 128],
                                    qT[:, isl], start=True, stop=True)
                            esb = phB2e.tile([128, 1024], f32r, tag="exp")
                            nc.scalar.activation(esb, psc, AF.Exp,
                                                 scale=ATT_SCALE)
                            m = J - 8 * c2
                            if m >= 0:
                                if m > 0:
                                    nc.vector.memset(
                                        esb[:, 0:m * 128].bitcast(f32), 0.0)
                                nc.vector.tensor_mul(
                                    esb[:, m * 128:(m + 1) * 128],
                                    esb[:, m * 128:(m + 1) * 128].bitcast(f32),
                                    tri_sb)
                            for half in range(2):
                                hs = slice(half * 512, (half + 1) * 512)
                                nc.tensor.matmul(psr[:, hs], onec_sb,
                                                 esb[:, hs],
                                                 start=(J == 0),
                                                 stop=(J == nj - 1))
                                nc.tensor.matmul(po[:, hs], vh[:, J, :],
                                                 esb[:, hs],
                                                 start=(J == 0),
                                                 stop=(J == nj - 1))
                        arow = phB2r.tile([1, 1024], f32, tag="arow")
                        nc.vector.tensor_copy(arow, psr)
                        nc.vector.reciprocal(arow, arow)
                        nc.sync.dma_start(out=ar_scr[:], in_=arow[:])
                        rbc2 = phB2e.tile([128, 1024], f32, tag="rbc2")
                        nc.sync.dma_start(
                            out=rbc2, in_=ar_scr[:].to_broadcast([128, 1024]))
                        ost = phB2e.tile([128, 1024], f32r, tag="ost")
                        nc.vector.tensor_mul(ost, po, rbc2)
                        nc.sync.dma_start(
                            out=oT_scr[h, :, c2 * 1024:(c2 + 1) * 1024],
                            in_=ost[:])

            # ---- phase C: output projection ----
            with tc.tile_pool(name="phC", bufs=1) as phC, \
                 tc.tile_pool(name="phCe", bufs=4) as phCe, \
                 tc.tile_pool(name="phC_ps", bufs=3, space="PSUM") as phC_ps:
                wo = phC.tile([128, HPC, DIM], f32r)
                for t in range(HPC):
                    nc.sync.dma_start(out=wo[:, t, :],
                                      in_=woT_d[t * 128:(t + 1) * 128, :])
                # wout column norms (free axis) combined with wv row norms
                sso = consts.tile([128, HPC], f32)
                for t in range(HPC):
                    sqo = phCe.tile([128, DIM], f32, tag="sqo")
                    nc.vector.tensor_mul(sqo, wo[:, t, :].bitcast(f32),
                                         wo[:, t, :].bitcast(f32))
                    nc.vector.tensor_reduce(
                        sso[:, t:t + 1], sqo, axis=mybir.AxisListType.X,
                        op=ALU.add)
                comb = consts.tile([128, HPC], f32)
                nc.scalar.activation(comb, sso, AF.Sqrt)
                nc.vector.reciprocal(comb, comb)
                nc.vector.tensor_mul(comb, comb, wv_rn)
                for t in range(HPC):
                    nc.vector.tensor_scalar_mul(
                        wo[:, t, :], wo[:, t, :].bitcast(f32), comb[:, t:t + 1])

                oT_all = phC.tile([128, HPC, SEQ], f32r)
                for h in range(HPC):
                    nc.sync.dma_start(out=oT_all[:, h, :], in_=oT_scr[h])

                for d in range(DIM // 128):
                    for c in range(NCH):
                        pP = phC_ps.tile([128, 512], f32, tag="pP")
                        for t in range(HPC):
                            nc.tensor.matmul(
                                pP, wo[:, t, d * 128:(d + 1) * 128],
                                oT_all[:, t, c * 512:(c + 1) * 512],
                                start=(t == 0), stop=(t == HPC - 1))
                        Psb = phCe.tile([128, 512], f32, tag="Pev")
                        if (d * NCH + c) % 2 == 0:
                            nc.vector.tensor_copy(Psb, pP)
                        else:
                            nc.scalar.copy(Psb, pP)
                        nc.sync.dma_start(
                            out=out_d[d * 128:(d + 1) * 128,
                                      c * 512:(c + 1) * 512],
                            in_=Psb[:])

    nc.compile()
    return nc


_CACHE = {}


def _get_program(repeat=1):
    if repeat not in _CACHE:
        _CACHE[repeat] = build_program(repeat)
    return _CACHE[repeat]


def _make_in_maps(x, Wq, Wk, Wv, Wout, qk_scale):
    tri = np.triu(np.ones((128, 128), dtype=np.float32))  # valid: i' >= j'
    onec = np.ones((128, 1), dtype=np.float32)
    in_maps = []
    for core in range(NCORES):
        b, g = divmod(core, HPC)
        sl = slice(g * ES, (g + 1) * ES)
        wq = Wq[sl]
        wk = Wk[sl]
        wv = Wv[sl]
        in_maps.append({
            "xT": np.ascontiguousarray(x[b].T),
            "wqT": np.ascontiguousarray(
                wq.T.reshape(DIM, HPC, DH).transpose(1, 0, 2)),
            "wkT": np.ascontiguousarray(
                wk.T.reshape(DIM, HPC, DH).transpose(1, 0, 2)),
            "wvT": np.ascontiguousarray(wv.T),
            "wqN": np.ascontiguousarray(wq.reshape(HPC, 128, DIM)),
            "wkN": np.ascontiguousarray(wk.reshape(HPC, 128, DIM)),
            "wvN": np.ascontiguousarray(wv.reshape(HPC, 128, DIM)),
            "woT": np.ascontiguousarray(Wout[:, sl].T),
            "qs": np.ascontiguousarray(qk_scale[sl].reshape(HPC, 128).T),
            "tri": tri,
            "onec": onec,
        })
    return in_maps


def _assemble(results):
    out = np.empty((B, SEQ, DIM), dtype=np.float32)
    for b in range(B):
        acc = results[4 * b]["out"].astype(np.float32).copy()
        for g in range(1, HPC):
            acc += results[4 * b + g]["out"]
        out[b] = acc.T
    return out


def kernel(x, Wq, Wk, Wv, Wout, qk_scale):
    nc = _get_program()
    in_maps = _make_in_maps(x, Wq, Wk, Wv, Wout, qk_scale)
    res = run_bass_kernel_spmd(nc, in_maps, core_ids=list(range(NCORES)))
    return _assemble(res.results)



# revision 12
# speedup vs baseline: 1.8294x; 1.8294x over previous
"""nGPT-style causal attention block on 8 TRN2 NeuronCores.

Sharding: core = (batch b, head-group g); b = core // 4, g = core % 4.
Each core handles 1 batch x 4 heads (512-channel slice) and produces a
partial P = l2norm_cols(Wout)[:, sl] @ oT of shape [DIM, SEQ] in bf16;
the host sums the 4 head-group partials per batch and transposes.

All tensors stay SBUF-resident in bf16 (no DRAM scratch round-trips).
Weight row/col norms run on ACT (in-place Square + accum_out over
natural layouts). Per-query q/k l2 norms use a ones-column matmul
partition reduction; reciprocal-norm row broadcasts across partitions
use rank-1 matmuls into PSUM (no SBUF->SBUF broadcast DMA). Causal
attention computes only valid key blocks with partial-width matmuls;
softmax needs no max pass (scores bounded by qk_scale * sqrt(dh)).
"""
import numpy as np
from contextlib import ExitStack

import concourse.bacc as bacc
import concourse.tile as tile
from concourse import mybir
from concourse.bass_utils import run_bass_kernel_spmd

DIM = 2048          # model dim (= contraction dim of projections)
SEQ = 2048          # sequence length
B = 2               # batch
HEADS = 16
DH = 128            # head dim
NCORES = 8
HPC = 4             # heads per core
ES = HPC * DH       # 512 channels per core
KT = DIM // 128     # 16 contraction tiles
NCH = SEQ // 512    # 4 query chunks of 512
ATT_SCALE = float(DH) ** 0.5

f32 = mybir.dt.float32
f32r = mybir.dt.float32r
bf16 = mybir.dt.bfloat16
AF = mybir.ActivationFunctionType
ALU = mybir.AluOpType


def build_program(repeat=1):
    nc = bacc.Bacc("TRN2", target_bir_lowering=False)

    # ---- per-core DRAM I/O ----
    xT_d = nc.dram_tensor("xT", [128, KT, SEQ], bf16, kind="ExternalInput")
    wqT_d = nc.dram_tensor("wqT", [128, KT, ES], bf16, kind="ExternalInput")
    wkT_d = nc.dram_tensor("wkT", [128, KT, ES], bf16, kind="ExternalInput")
    wvT_d = nc.dram_tensor("wvT", [128, KT, ES], bf16, kind="ExternalInput")
    wqN_d = nc.dram_tensor("wqN", [128, HPC, DIM], bf16, kind="ExternalInput")
    wkN_d = nc.dram_tensor("wkN", [128, HPC, DIM], bf16, kind="ExternalInput")
    wvN_d = nc.dram_tensor("wvN", [128, HPC, DIM], bf16, kind="ExternalInput")
    woT_d = nc.dram_tensor("woT", [128, HPC, DIM], bf16, kind="ExternalInput")
    se_d = nc.dram_tensor("se", [1, ES], f32r, kind="ExternalInput")
    onesr_d = nc.dram_tensor("onesr", [1, 128], f32r, kind="ExternalInput")
    onec_d = nc.dram_tensor("onec", [128, 1], f32r, kind="ExternalInput")
    onecb_d = nc.dram_tensor("onecb", [128, 1], bf16, kind="ExternalInput")
    tri_d = nc.dram_tensor("tri", [128, 128], bf16, kind="ExternalInput")
    out_d = nc.dram_tensor("out", [DIM, SEQ], bf16, kind="ExternalOutput")

    with tile.TileContext(nc) as tc:
      for _rep in range(repeat):
        with ExitStack() as top:
            consts = top.enter_context(tc.tile_pool(name="consts", bufs=1))
            scr = top.enter_context(tc.tile_pool(name="scr", bufs=1,
                                                 space="DRAM"))
            persist = top.enter_context(tc.tile_pool(name="persist", bufs=1))

            # persistent activation stores (bf16)
            v_sb = persist.tile([128, KT, ES], bf16, tag="v")
            q_sb = persist.tile([128, HPC, SEQ], bf16, tag="q")
            k_sb = persist.tile([128, HPC, SEQ], bf16, tag="k")
            oT_sb = persist.tile([128, HPC, SEQ], bf16, tag="oT")

            tri_sb = consts.tile([128, 128], bf16, tag="tri")
            se_sb = consts.tile([1, ES], f32r, tag="se")
            onesr = consts.tile([1, 128], f32r, tag="onesr")
            onec = consts.tile([128, 1], f32r, tag="onec")
            onecb = consts.tile([128, 1], bf16, tag="onecb")
            rn_q = consts.tile([128, HPC], f32, tag="rn_q")
            rn_k = consts.tile([128, HPC], f32, tag="rn_k")
            rn_v = consts.tile([128, HPC], f32, tag="rn_v")
            comb = consts.tile([128, HPC], f32r, tag="comb")
            comb_row = consts.tile([1, ES], f32r, tag="comb_row")
            ssn_q = consts.tile([128, HPC], f32, tag="ssn_q")
            ssn_k = consts.tile([128, HPC], f32, tag="ssn_k")
            ssn_v = consts.tile([128, HPC], f32, tag="ssn_v")
            sso = consts.tile([128, HPC], f32, tag="sso")
            comb_scr = scr.tile([128, HPC], f32r)

            nc.sync.dma_start(out=tri_sb, in_=tri_d[:])
            nc.sync.dma_start(out=se_sb, in_=se_d[:])
            nc.sync.dma_start(out=onesr, in_=onesr_d[:])
            nc.sync.dma_start(out=onec, in_=onec_d[:])
            nc.sync.dma_start(out=onecb, in_=onecb_d[:])

            # ---- weight tiles: wq/wk/wv rotate through 2 slots ----
            wts = top.enter_context(tc.tile_pool(name="wts", bufs=2))
            wop = top.enter_context(tc.tile_pool(name="wop", bufs=1))
            wq_sb = wts.tile([128, KT, ES], bf16, tag="w")
            wk_sb = wts.tile([128, KT, ES], bf16, tag="w")
            wo_sb = wop.tile([128, HPC, DIM], bf16, tag="wo")
            nc.sync.dma_start(out=wq_sb, in_=wqT_d[:])
            nc.sync.dma_start(out=wk_sb, in_=wkT_d[:])
            nc.sync.dma_start(out=wo_sb, in_=woT_d[:])

            with ExitStack() as xctx:
                # ---- x resident (per k-tile DMAs so matmuls start early) --
                xpool = xctx.enter_context(tc.tile_pool(name="xpool", bufs=1))
                xt = xpool.tile([128, KT, SEQ], bf16)
                for k in range(KT):
                    nc.sync.dma_start(out=xt[:, k, :], in_=xT_d[:, k, :])

                # ---- weight norms: in-place ACT Square + accum_out ----
                # rn = 1/sqrt(sum_d W^2) per channel, partition-major [128,4]
                with tc.tile_pool(name="wnat", bufs=1) as wnat:
                    for w_nat_d, ssn, rn in ((wqN_d, ssn_q, rn_q),
                                             (wkN_d, ssn_k, rn_k),
                                             (wvN_d, ssn_v, rn_v)):
                        nt = wnat.tile([128, HPC, DIM], bf16, tag="nat")
                        nc.sync.dma_start(out=nt, in_=w_nat_d[:])
                        for t in range(HPC):
                            nc.scalar.activation(nt[:, t, :], nt[:, t, :],
                                                 AF.Square,
                                                 accum_out=ssn[:, t:t + 1])
                        nc.vector.reciprocal(rn, ssn)
                        nc.scalar.activation(rn, rn, AF.Sqrt)
                    # wout col norms from woT (free axis) -> fold in v norms
                    sqo = wnat.tile([128, HPC, DIM], bf16, tag="nat")
                    nc.vector.tensor_copy(sqo, wo_sb)
                    for t in range(HPC):
                        nc.scalar.activation(sqo[:, t, :], sqo[:, t, :],
                                             AF.Square,
                                             accum_out=sso[:, t:t + 1])
                    nc.vector.reciprocal(comb.bitcast(f32), sso)
                    nc.scalar.activation(comb.bitcast(f32),
                                         comb.bitcast(f32), AF.Sqrt)
                    # final producer writes f32r (rounds for the PE fast path)
                    nc.vector.tensor_mul(comb, comb.bitcast(f32), rn_v)
                    # transpose comb [128,4] -> comb_row [1,512] via DRAM hop
                    nc.sync.dma_start(out=comb_scr, in_=comb[:])
                    for t in range(HPC):
                        nc.sync.dma_start(
                            out=comb_row[:, t * 128:(t + 1) * 128],
                            in_=comb_scr[:, t:t + 1])

                # ====== phase B: q/k projections + per-query l2 norms ======
                with tc.tile_pool(name="qkps", bufs=2, space="PSUM") as qkps, \
                     tc.tile_pool(name="nrm_ps", bufs=2,
                                  space="PSUM") as nrm_ps, \
                     tc.tile_pool(name="brow", bufs=2) as brow, \
                     tc.tile_pool(name="sqp", bufs=2) as sqp:
                    for h in range(HPC):
                        for dst, w_sb, rn, is_q in ((q_sb, wq_sb, rn_q, True),
                                                    (k_sb, wk_sb, rn_k,
                                                     False)):
                            for half in range(2):
                                qps = qkps.tile([128, 1024], f32, tag="qps")
                                for k in range(KT):
                                    for j in range(2):
                                        sl = slice(half * 1024 + j * 512,
                                                   half * 1024 + (j + 1) * 512)
                                        nc.tensor.matmul(
                                            qps[:, j * 512:(j + 1) * 512],
                                            w_sb[:, k, h * 128:(h + 1) * 128],
                                            xt[:, k, sl],
                                            start=(k == 0), stop=(k == KT - 1))
                                # evict with weight-row-norm fold -> bf16
                                nc.vector.tensor_scalar_mul(
                                    dst[:, h, half * 1024:(half + 1) * 1024],
                                    qps, rn[:, h:h + 1])
                            for c in range(NCH):
                                cs = slice(c * 512, (c + 1) * 512)
                                sq = sqp.tile([128, 512], f32r, tag="sq")
                                nc.vector.tensor_mul(
                                    sq, dst[:, h, cs], dst[:, h, cs])
                                ssb = nrm_ps.tile([1, 512], f32, tag="ss")
                                nc.tensor.matmul(ssb, onec, sq,
                                                 start=True, stop=True)
                                srow = brow.tile([1, 512], f32, tag="srow")
                                nc.scalar.activation(srow, ssb, AF.Sqrt)
                                rrow = brow.tile([1, 512], f32r, tag="rrow")
                                with nc.allow_low_precision(
                                        reason="f32r is full-width storage"):
                                    nc.vector.reciprocal(rrow, srow)
                                bc = nrm_ps.tile([128, 512], f32, tag="bc")
                                lhs = (se_sb[:, h * 128:(h + 1) * 128] if is_q
                                       else onesr[:, :])
                                nc.tensor.matmul(bc, lhs, rrow,
                                                 start=True, stop=True)
                                nc.vector.tensor_mul(
                                    dst[:, h, cs], dst[:, h, cs], bc)

                # ============ phase A: v projection (natural) ==============
                wv_sb = wts.tile([128, KT, ES], bf16, tag="w")
                nc.sync.dma_start(out=wv_sb, in_=wvT_d[:])
                with tc.tile_pool(name="vps", bufs=3, space="PSUM") as vps_p:
                    for t in range(SEQ // 128):
                        pv = vps_p.tile([128, ES], f32, tag="pv")
                        for k in range(KT):
                            nc.tensor.matmul(
                                pv, xt[:, k, t * 128:(t + 1) * 128],
                                wv_sb[:, k, :],
                                start=(k == 0), stop=(k == KT - 1))
                        nc.vector.tensor_copy(v_sb[:, t, :], pv)

            # ================= phase C: causal attention ===================
            with tc.tile_pool(name="po_ps", bufs=2, space="PSUM") as po_ps, \
                 tc.tile_pool(name="psr_ps", bufs=2, space="PSUM") as psr_ps, \
                 tc.tile_pool(name="psc_ps", bufs=2, space="PSUM") as psc_ps, \
                 tc.tile_pool(name="cbc_ps", bufs=2, space="PSUM") as cbc_ps, \
                 tc.tile_pool(name="epool", bufs=3) as epool, \
                 tc.tile_pool(name="crow", bufs=2) as crow_pool, \
                 tc.tile_pool(name="cbs", bufs=2) as cbs_pool:
                for h in range(HPC):
                    for c in range(NCH):
                        po = po_ps.tile([128, 512], f32, tag="po")
                        psr = psr_ps.tile([1, 512], f32, tag="psr")
                        nj = 4 * c + 4
                        for J in range(nj):
                            m = J - 4 * c
                            lo = max(m, 0) * 128
                            width = 512 - lo
                            psc = psc_ps.tile([128, 512], f32, tag="psc")
                            nc.tensor.matmul(
                                psc[:, :width],
                                k_sb[:, h, J * 128:(J + 1) * 128],
                                q_sb[:, h, c * 512 + lo:(c + 1) * 512],
                                start=True, stop=True)
                            esb = epool.tile([128, 512], bf16, tag="esb")
                            nc.scalar.activation(esb[:, :width],
                                                 psc[:, :width], AF.Exp,
                                                 scale=ATT_SCALE)
                            if m >= 0:
                                # diagonal 128-block: upper-tri (q>=k) mask
                                nc.vector.tensor_mul(
                                    esb[:, 0:128], esb[:, 0:128], tri_sb)
                            nc.tensor.matmul(psr[:, lo:], onecb,
                                             esb[:, :width],
                                             start=(J == 0),
                                             stop=(J == nj - 1))
                            nc.tensor.matmul(po[:, lo:],
                                             v_sb[:, J, h * 128:(h + 1) * 128],
                                             esb[:, :width],
                                             start=(J == 0),
                                             stop=(J == nj - 1))
                        rrow = crow_pool.tile([1, 512], f32r, tag="crow")
                        with nc.allow_low_precision(
                                reason="f32r is full-width storage"):
                            nc.vector.reciprocal(rrow, psr)
                        bc = cbc_ps.tile([128, 512], f32, tag="cbc")
                        nc.tensor.matmul(
                            bc, comb_row[:, h * 128:(h + 1) * 128],
                            rrow, start=True, stop=True)
                        bcs = cbs_pool.tile([128, 512], f32, tag="bcs")
                        nc.vector.tensor_copy(bcs, bc)
                        nc.vector.tensor_mul(
                            oT_sb[:, h, c * 512:(c + 1) * 512], po, bcs)

            # ================= phase D: output projection ==================
            with tc.tile_pool(name="d_ps", bufs=2, space="PSUM") as d_ps, \
                 tc.tile_pool(name="opool", bufs=2) as opool:
                for d in range(DIM // 128):
                    dps = d_ps.tile([128, SEQ], f32, tag="dps")
                    for t in range(HPC):
                        for j in range(NCH):
                            nc.tensor.matmul(
                                dps[:, j * 512:(j + 1) * 512],
                                wo_sb[:, t, d * 128:(d + 1) * 128],
                                oT_sb[:, t, j * 512:(j + 1) * 512],
                                start=(t == 0), stop=(t == HPC - 1))
                    ob = opool.tile([128, SEQ], bf16, tag="ob")
                    if d % 2 == 0:
                        nc.vector.tensor_copy(ob, dps)
                    else:
                        nc.scalar.copy(ob, dps)
                    nc.sync.dma_start(
                        out=out_d[d * 128:(d + 1) * 128, :], in_=ob[:])

    nc.compile()
    return nc


_CACHE = {}


def _get_program(repeat=1):
    if repeat not in _CACHE:
        _CACHE[repeat] = build_program(repeat)
    return _CACHE[repeat]


def _make_in_maps(x, Wq, Wk, Wv, Wout, qk_scale):
    nbf = mybir.dt.np(bf16)
    tri = np.triu(np.ones((128, 128), dtype=np.float32)).astype(nbf)
    onec = np.ones((128, 1), dtype=np.float32)
    onecb = np.ones((128, 1), dtype=np.float32).astype(nbf)
    onesr = np.ones((1, 128), dtype=np.float32)

    def t3(a, kt, p, n):  # [kt*p, n] -> [p, kt, n]
        return np.ascontiguousarray(
            a.reshape(kt, p, n).transpose(1, 0, 2)).astype(nbf)

    in_maps = []
    for core in range(NCORES):
        b, g = divmod(core, HPC)
        sl = slice(g * ES, (g + 1) * ES)
        wq, wk, wv = Wq[sl], Wk[sl], Wv[sl]
        in_maps.append({
            "xT": t3(x[b].T, KT, 128, SEQ),
            "wqT": t3(wq.T, KT, 128, ES),
            "wkT": t3(wk.T, KT, 128, ES),
            "wvT": t3(wv.T, KT, 128, ES),
            "wqN": t3(wq, HPC, 128, DIM),
            "wkN": t3(wk, HPC, 128, DIM),
            "wvN": t3(wv, HPC, 128, DIM),
            "woT": t3(Wout[:, sl].T, HPC, 128, DIM),
            "se": np.ascontiguousarray(
                (qk_scale[sl] * DIM).reshape(1, ES)).astype(np.float32),
            "onesr": onesr,
            "onec": onec,
            "onecb": onecb,
            "tri": tri,
        })
    return in_maps


def _assemble(results):
    out = np.empty((B, SEQ, DIM), dtype=np.float32)
    for b in range(B):
        acc = results[4 * b]["out"].astype(np.float32)
        for g in range(1, HPC):
            acc = acc + results[4 * b + g]["out"].astype(np.float32)
        out[b] = acc.T
    return out


def kernel(x, Wq, Wk, Wv, Wout, qk_scale):
    nc = _get_program()
    in_maps = _make_in_maps(x, Wq, Wk, Wv, Wout, qk_scale)
    res = run_bass_kernel_spmd(nc, in_maps, core_ids=list(range(NCORES)))
    return _assemble(res.results)
